# revision 11
# baseline (speedup 1.0000x reference)
"""Circular rational-quadratic spline flow on 8 Trainium2 cores.

Data-parallel over the batch (131072 rows -> 16384/core). Per core:
MLP on PE (f32r matmuls, relu via ACT), spline in a rows-on-partitions
layout. Bin search + all gathers are done with masked tensor_tensor_scan
tails: state=(data+state)*W, where complement masks W zero the state at
segment boundaries, so one scan over a [kill|e_1..e_32] x 8d stream
yields per-(row,d) tail sums whose differences give every gathered
quantity.

v2 changes vs baseline:
- all matmuls use float32r (1 cycle/col at N>=256 vs 4 for float32)
- the 7 masked tail scans are merged into 3 scan instructions
  (e-pair, f-pair, u-triple) via an extra leading AP dim; sub-stream
  boundaries are killed by the wx zero-pads, so one running state is safe
- work split across engines: f-pair scan, mj/r2, zw/zh reduces and part
  of the group-tail math run on GpSimd (Pool), freeing DVE
"""

import dataclasses
import numpy as np

import concourse.bacc as bacc
import concourse.mybir as mybir
import concourse.tile as tile
from concourse.bass_utils import run_bass_kernel_spmd

TWO_PI = 2.0 * np.pi
MIN_W = 1e-3
MIN_H = 1e-3
MIN_D = 1e-3
DERIV_SHIFT = float(np.log(np.e - 1.0))
K = 32
DH = 8
C = 64
H = 256
NCORES = 8

F32 = mybir.dt.float32
F32R = mybir.dt.float32r
ALU = mybir.AluOpType
AX = mybir.AxisListType
ACTF = mybir.ActivationFunctionType


def _ap(ap, dims, offset_elems=0, partitions=None):
    """AP with explicit free dims [(step, count), ...] in elements."""
    p = ap.ap[0]
    if partitions is not None:
        p = [p[0], partitions]
    aps = [p] + [[s, c] for (s, c) in dims]
    return dataclasses.replace(ap, ap=aps, offset=ap.offset + offset_elems)


def _scan_raw(nc, eng, out, data0, data1):
    """tensor_tensor_scan with multi-free-dim APs (bypasses 2D assert).
    state = (data0 + state) * data1 in AP stream order."""
    return eng.add_instruction(
        mybir.InstTensorScalarPtr(
            name=nc.get_next_instruction_name(),
            is_tensor_tensor_scan=True,
            is_scalar_tensor_tensor=True,
            op0=ALU.add,
            op1=ALU.mult,
            ins=[
                eng.lower_ap(data0),
                eng.lower_ap_or_imm(0.0),
                eng.lower_ap(data1),
            ],
            outs=[eng.lower_ap(out)],
        )
    )


def _r(ap_obj):
    """matmul input tiles are already float32r."""
    return ap_obj


_NC_CACHE = {}


def build_kernel(b_core):
    if b_core in _NC_CACHE:
        return _NC_CACHE[b_core]
    NQ = b_core // 128          # number of 128-row batch chunks
    GQ = min(32, NQ)            # chunks per tail group
    assert NQ % GQ == 0
    NG = NQ // GQ
    a_w = 1.0 - MIN_W * K
    a_h = 1.0 - MIN_H * K

    nc = bacc.Bacc("TRN2", debug=False)
    theta_d = nc.dram_tensor("theta", [b_core, DH], F32, kind="ExternalInput")
    xT_d = nc.dram_tensor("xT", [C, b_core], F32R, kind="ExternalInput")
    w1_d = nc.dram_tensor("w1", [C, H], F32R, kind="ExternalInput")
    b1_d = nc.dram_tensor("b1", [128, 2], F32, kind="ExternalInput")
    w2_d = nc.dram_tensor("w2", [H + 1, 776], F32R, kind="ExternalInput")
    basis_d = nc.dram_tensor("basis", [16, 256], F32R, kind="ExternalInput")
    ident_d = nc.dram_tensor("ident", [128, 128], F32, kind="ExternalInput")
    out_d = nc.dram_tensor("outs", [b_core, DH], F32, kind="ExternalOutput")
    lad_d = nc.dram_tensor("lad", [b_core, DH], F32, kind="ExternalOutput")

    with tile.TileContext(nc) as tc:
        with tc.tile_pool(name="const", bufs=1) as cpool, \
             tc.tile_pool(name="w2p", bufs=1) as wpool, \
             tc.tile_pool(name="mm1", bufs=2) as mpool, \
             tc.tile_pool(name="mm1ps", bufs=1, space="PSUM") as mm1ps, \
             tc.tile_pool(name="prps", bufs=2, space="PSUM") as prps, \
             tc.tile_pool(name="r2ps", bufs=2, space="PSUM") as r2pool, \
             tc.tile_pool(name="trps", bufs=1, space="PSUM") as trpool, \
             tc.tile_pool(name="chunk", bufs=8) as kpool, \
             tc.tile_pool(name="grp", bufs=2) as gpool, \
             tc.tile_pool(name="gtmp", bufs=1) as tpool:

            # ---------------- resident constants ----------------
            w1_t = cpool.tile([C, H], F32R)
            nc.sync.dma_start(w1_t[:, :], w1_d.ap())
            b1_t = cpool.tile([128, 2], F32)
            nc.sync.dma_start(b1_t[:, :], b1_d.ap())
            w2_t = wpool.tile([128, 2, 776], F32R)
            nc.sync.dma_start(
                w2_t[:, :, :],
                _ap(w2_d.ap(), [(776 * 128, 2), (1, 776)], partitions=128))
            b2row_t = cpool.tile([1, 776], F32R)
            nc.sync.dma_start(
                b2row_t[:, :],
                _ap(w2_d.ap(), [(1, 776)], offset_elems=256 * 776, partitions=1))
            ones_t = cpool.tile([1, 128], F32R)
            nc.vector.memset(ones_t[:, :].bitcast(F32), 1.0)
            basis_t = cpool.tile([16, 256], F32R)
            nc.sync.dma_start(basis_t[:, :], basis_d.ap())
            ident_t = cpool.tile([128, 128], F32)
            nc.sync.dma_start(ident_t[:, :], ident_d.ap())
            # per-partition bias constants for ACT-side affine tail ops
            cb_t = cpool.tile([128, 4], F32)
            nc.vector.memset(cb_t[:, 0:1], TWO_PI * MIN_W * K)
            nc.vector.memset(cb_t[:, 1:2], TWO_PI * MIN_W)
            nc.vector.memset(cb_t[:, 2:3], MIN_D)
            nc.vector.memset(cb_t[:, 3:4], 1.0)
            # carry-kill mask for the plain cumsum: 0 at stream col 33*d
            wc_t = cpool.tile([128, 264], F32)
            nc.vector.memset(wc_t[:, :], 1.0)
            nc.vector.memset(_ap(wc_t[:, :], [(33, DH), (1, 1)]), 0.0)
            # persistent ring buffers for ef / wx so constant pads are
            # written once instead of per chunk
            NB = 8
            # ef layout per buf: [pad(1) | e(256) | f(256) | pad(1)]; the e/f
            # streams then form one uniform 16-segment [(32,16),(1,33)] AP
            # (f seg d uses e's last element as its kill slot)
            efr_t = cpool.tile([128, NB, 514], F32)
            nc.vector.memset(_ap(efr_t[:, :, :], [(1, NB * 514)]), 1.0)
            wxr_t = cpool.tile([128, NB, 16, 36], F32)
            nc.vector.memset(_ap(wxr_t[:, :, :, :], [(1, NB * 16 * 36)]), 0.0)
            nc.vector.memset(
                _ap(wxr_t[:, :, :, :], [(36, NB * 16), (1, 1)], offset_elems=35),
                1.0)
            # 16-segment kill mask for the merged s/zh cumsum scan
            wc16_t = cpool.tile([128, 528], F32)
            nc.vector.memset(wc16_t[:, :], 1.0)
            nc.vector.memset(_ap(wc16_t[:, :], [(33, 16), (1, 1)]), 0.0)

            for g in range(NG):
                th_g = gpool.tile([128, GQ, DH], F32, tag="th")
                zw_g = gpool.tile([128, GQ, DH], F32, tag="zw")
                zh_g = gpool.tile([128, GQ, DH], F32, tag="zh")
                ix_g = gpool.tile([128, GQ, DH], F32, tag="ix")
                tl_g = gpool.tile([128, GQ, 7, DH], F32, tag="tl")
                nc.sync.dma_start(
                    th_g[:, :, :],
                    _ap(theta_d.ap(), [(128 * DH, GQ), (1, DH)],
                        offset_elems=g * GQ * 128 * DH, partitions=128))

                for qq in range(GQ):
                    q = g * GQ + qq
                    ql = q % 4
                    if ql == 0:
                        # -------- MM1 for the next 512 batch rows --------
                        xt_t = mpool.tile([C, 512], F32R, tag="xt")
                        nc.sync.dma_start(
                            xt_t[:, :],
                            _ap(xT_d.ap(), [(1, 512)], offset_elems=q * 128))
                        ht_t = mpool.tile([128, 2, 512], F32R, tag="ht")
                        for kh in range(2):
                            hps = mm1ps.tile([128, 512], F32, tag="hps")
                            nc.tensor.matmul(hps[:, :],
                                             _r(w1_t[:, 128 * kh:128 * (kh + 1)]),
                                             _r(xt_t[:, :]), start=True, stop=True)
                            nc.scalar.activation(ht_t[:, kh, :], hps[:, :],
                                                 ACTF.Relu,
                                                 bias=b1_t[:, kh:kh + 1],
                                                 scale=1.0)
                    # -------- MM2: params chunk -> PSUM [128, 776] --------
                    pps = prps.tile([128, 1024], F32, tag="pps")
                    for kh in range(2):
                        lhs = _r(ht_t[:, kh, 128 * ql:128 * (ql + 1)])
                        nc.tensor.matmul(pps[:, 0:512], lhs, _r(w2_t[:, kh, 0:512]),
                                         start=(kh == 0), stop=False)
                        nc.tensor.matmul(pps[:, 512:776], lhs,
                                         _r(w2_t[:, kh, 512:776]),
                                         start=(kh == 0), stop=False)
                    nc.tensor.matmul(pps[:, 0:512], _r(ones_t[:, :]),
                                     _r(b2row_t[:, 0:512]), start=False, stop=True)
                    nc.tensor.matmul(pps[:, 512:776], _r(ones_t[:, :]),
                                     _r(b2row_t[:, 512:776]), start=False, stop=True)

                    # -------- spline chunk --------
                    # EF[:, a, 1+32d+k] = exp(params)  (a=0: uw, a=1: uh)
                    ef_t = efr_t[:, q % NB]
                    nc.scalar.activation(
                        _ap(ef_t[:, :], [(256, 2), (1, 256)], offset_elems=1),
                        pps[:, 0:512], ACTF.Exp, scale=1.0)
                    # merged cumsum over 16 segments: S (e, segs 0-7) and
                    # cum-f (segs 8-15); Zw/Zh are the segment-end values
                    s_t = kpool.tile([128, 528], F32, tag="s")
                    _scan_raw(nc, nc.vector, s_t[:, :],
                              _ap(ef_t[:, :], [(32, 16), (1, 33)]),
                              wc16_t[:, :])
                    nc.gpsimd.tensor_copy(
                        zw_g[:, qq, :],
                        _ap(s_t[:, :], [(33, DH)], offset_elems=32))
                    nc.gpsimd.tensor_copy(
                        zh_g[:, qq, :],
                        _ap(s_t[:, :], [(33, DH)], offset_elems=264 + 32))
                    # smalls = [Tp | Zw] per row; R2 = basis.T-combo on PE:
                    # R2[row, 32d+k] = Tp[row,d] - (k+1)*MIN_W/a_w*Zw[row,d]
                    sm_t = kpool.tile([128, 16], F32, tag="sm")
                    nc.vector.scalar_tensor_tensor(
                        sm_t[:, 0:DH], th_g[:, qq, :], 1.0 / (TWO_PI * a_w),
                        zw_g[:, qq, :], ALU.mult, ALU.mult)
                    nc.gpsimd.tensor_copy(
                        sm_t[:, DH:16],
                        _ap(s_t[:, :], [(33, DH)], offset_elems=32))
                    smt_ps = trpool.tile([16, 128], F32, tag="smt")
                    nc.tensor.transpose(smt_ps[:, :], sm_t[:, :], ident_t[:, :])
                    smt_t = kpool.tile([16, 128], F32R, tag="smts")
                    nc.scalar.activation(smt_t[:, :], smt_ps[:, :],
                                         ACTF.Copy, scale=1.0)
                    r2_ps = r2pool.tile([128, 256], F32, tag="r2")
                    nc.tensor.matmul(r2_ps[:, :], smt_t[:, :], basis_t[:, :],
                                     start=True, stop=True)
                    r2_t = kpool.tile([128, 256], F32, tag="r2s")
                    nc.scalar.activation(r2_t[:, :], r2_ps[:, :],
                                         ACTF.Copy, scale=1.0)
                    # Wext[d, c] = W_{c-2} = [S_local > R2]; c in {0,1,2}->0, 35->1
                    wx_t = wxr_t[:, q % NB]
                    nc.vector.tensor_tensor(
                        _ap(wx_t[:, :, :], [(36, DH), (1, K)], offset_elems=3),
                        _ap(s_t[:, :], [(33, DH), (1, K)], offset_elems=1),
                        _ap(r2_t[:, :], [(K, DH), (1, K)]), ALU.is_gt)
                    nc.gpsimd.tensor_copy(
                        _ap(wx_t[:, :, :], [(36, DH), (1, K)],
                            offset_elems=36 * DH + 3),
                        _ap(wx_t[:, :, :], [(36, DH), (1, K)], offset_elems=3))
                    # idx = 32 - sum(W)
                    nc.vector.tensor_reduce(
                        ix_g[:, qq, :],
                        _ap(wx_t[:, :, :], [(36, DH), (1, K)], offset_elems=3),
                        axis=AX.X, op=ALU.add)
                    # ud params psum -> SBUF (GPSIMD cannot read PSUM)
                    ud_t = kpool.tile([128, 265], F32, tag="ud")
                    nc.scalar.activation(ud_t[:, :], pps[:, 511:776],
                                         ACTF.Copy, scale=1.0)
                    # -------- 7 masked tail scans --------
                    # out APs overlap with stride 0 along the stream, so each
                    # d-segment's 33/34 running values land on one address and
                    # the final value (the masked tail) survives -> tails are
                    # written directly into the group tile, no scratch/copies
                    # merged e+f tails (16 segs -> tl slots t_i, t_i+1)
                    for (t_i, sh) in ((0, 0), (2, 1)):
                        _scan_raw(
                            nc, nc.vector,
                            _ap(tl_g[:, :, :, :], [(1, 16), (0, 33)],
                                offset_elems=(qq * 7 + t_i) * DH),
                            _ap(ef_t[:, :], [(32, 16), (1, 33)]),
                            _ap(wx_t[:, :, :], [(36, 16), (1, 33)],
                                offset_elems=2 - sh))
                    for (t_i, sh) in ((4, 0), (5, 1), (6, 2)):
                        _scan_raw(
                            nc, nc.vector,
                            _ap(tl_g[:, :, :, :], [(1, DH), (0, 34)],
                                offset_elems=(qq * 7 + t_i) * DH),
                            _ap(ud_t[:, :], [(33, DH), (1, 34)]),
                            _ap(wx_t[:, :, :], [(36, DH), (1, 34)],
                                offset_elems=2 - sh))

                # ============ group tail: per-row rational quadratic ============
                def gt(tag):
                    return tpool.tile([128, GQ, DH], F32, tag=tag, name=tag)

                def f2(t):
                    return t[:, :, :]

                TT = nc.vector.tensor_tensor
                STT = nc.vector.scalar_tensor_tensor
                PTT = nc.vector.tensor_tensor
                PSTT = nc.vector.scalar_tensor_tensor
                rzw = gt("rzw"); rzh = gt("rzh")
                nc.vector.reciprocal(f2(rzw), f2(zw_g))
                nc.vector.reciprocal(f2(rzh), f2(zh_g))
                sm0 = gt("sm0"); sm1 = gt("sm1"); hm0 = gt("hm0"); hm1 = gt("hm1")
                PTT(f2(sm0), f2(zw_g), tl_g[:, :, 0, :], ALU.subtract)
                PTT(f2(sm1), f2(zw_g), tl_g[:, :, 2, :], ALU.subtract)
                PTT(f2(hm0), f2(zh_g), tl_g[:, :, 1, :], ALU.subtract)
                PTT(f2(hm1), f2(zh_g), tl_g[:, :, 3, :], ALU.subtract)
                u1 = gt("u1"); u2 = gt("u2")
                PTT(f2(u1), tl_g[:, :, 4, :], tl_g[:, :, 5, :], ALU.subtract)
                PTT(f2(u2), tl_g[:, :, 5, :], tl_g[:, :, 6, :], ALU.subtract)
                icw = gt("icw"); inw = gt("inw"); ich = gt("ich"); inh = gt("inh")
                tmp = gt("tmp"); tmp2 = gt("tmp2")
                STT(f2(tmp), f2(sm0), TWO_PI * a_w, f2(rzw), ALU.mult, ALU.mult)
                STT(f2(icw), f2(ix_g), -TWO_PI * MIN_W, f2(tmp), ALU.mult, ALU.add)
                nc.scalar.activation(f2(icw), f2(icw), ACTF.Identity, bias=cb_t[:, 0:1])
                PTT(f2(tmp2), f2(sm1), f2(sm0), ALU.subtract)
                STT(f2(tmp), f2(tmp2), TWO_PI * a_w, f2(rzw), ALU.mult, ALU.mult)
                nc.scalar.activation(f2(inw), f2(tmp), ACTF.Identity, bias=cb_t[:, 1:2])
                PSTT(f2(tmp), f2(hm0), TWO_PI * a_h, f2(rzh), ALU.mult, ALU.mult)
                PSTT(f2(ich), f2(ix_g), -TWO_PI * MIN_H, f2(tmp), ALU.mult, ALU.add)
                nc.scalar.activation(f2(ich), f2(ich), ACTF.Identity, bias=cb_t[:, 0:1])
                PTT(f2(tmp2), f2(hm1), f2(hm0), ALU.subtract)
                PSTT(f2(tmp), f2(tmp2), TWO_PI * a_h, f2(rzh), ALU.mult, ALU.mult)
                nc.scalar.activation(f2(inh), f2(tmp), ACTF.Identity, bias=cb_t[:, 1:2])
                # d0/d1 = MIN_D + ln(1 + exp(u))
                e1 = gt("e1"); e2 = gt("e2"); dd0 = gt("dd0"); dd1 = gt("dd1")
                nc.scalar.activation(f2(e1), f2(u1), ACTF.Exp, scale=1.0)
                nc.scalar.activation(f2(e2), f2(u2), ACTF.Exp, scale=1.0)
                nc.scalar.activation(f2(dd0), f2(e1), ACTF.Ln, bias=cb_t[:, 3:4],
                                     scale=1.0)
                nc.scalar.activation(f2(dd1), f2(e2), ACTF.Ln, bias=cb_t[:, 3:4],
                                     scale=1.0)
                nc.scalar.activation(f2(dd0), f2(dd0), ACTF.Identity, bias=cb_t[:, 2:3])
                nc.scalar.activation(f2(dd1), f2(dd1), ACTF.Identity, bias=cb_t[:, 2:3])
                rw = gt("rw"); tt_ = gt("tt"); t1 = gt("t1")
                nc.vector.reciprocal(f2(rw), f2(inw))
                TT(f2(tmp), f2(th_g), f2(icw), ALU.subtract)
                TT(f2(tt_), f2(tmp), f2(rw), ALU.mult)
                nc.scalar.activation(f2(tmp), f2(tt_), ACTF.Identity,
                                     bias=cb_t[:, 3:4], scale=-1.0)   # 1 - t
                TT(f2(t1), f2(tt_), f2(tmp), ALU.mult)
                dl = gt("dl"); t2 = gt("t2"); omt2 = gt("omt2")
                PTT(f2(dl), f2(inh), f2(rw), ALU.mult)
                TT(f2(t2), f2(tt_), f2(tt_), ALU.mult)
                PTT(f2(omt2), f2(tmp), f2(tmp), ALU.mult)
                nm = gt("nm"); dn = gt("dn")
                TT(f2(tmp2), f2(dl), f2(t2), ALU.mult)
                TT(f2(nm), f2(dd0), f2(t1), ALU.mult)
                TT(f2(nm), f2(nm), f2(tmp2), ALU.add)
                TT(f2(nm), f2(nm), f2(inh), ALU.mult)
                PTT(f2(dn), f2(dd0), f2(dd1), ALU.add)
                STT(f2(dn), f2(dl), -2.0, f2(dn), ALU.mult, ALU.add)
                TT(f2(dn), f2(dn), f2(t1), ALU.mult)
                TT(f2(dn), f2(dn), f2(dl), ALU.add)
                rdn = gt("rdn"); outv = gt("outv")
                nc.vector.reciprocal(f2(rdn), f2(dn))
                TT(f2(outv), f2(nm), f2(rdn), ALU.mult)
                TT(f2(outv), f2(outv), f2(ich), ALU.add)
                dv = gt("dv")
                PTT(f2(dv), f2(dd1), f2(t2), ALU.mult)
                PSTT(f2(tmp2), f2(dl), 2.0, f2(t1), ALU.mult, ALU.mult)
                PTT(f2(dv), f2(dv), f2(tmp2), ALU.add)
                PTT(f2(tmp2), f2(dd0), f2(omt2), ALU.mult)
                PTT(f2(dv), f2(dv), f2(tmp2), ALU.add)
                PTT(f2(tmp2), f2(dl), f2(dl), ALU.mult)
                PTT(f2(dv), f2(dv), f2(tmp2), ALU.mult)
                ldv = gt("ldv"); ldn = gt("ldn"); ladv = gt("ladv")
                nc.scalar.activation(f2(ldv), f2(dv), ACTF.Ln, scale=1.0)
                nc.scalar.activation(f2(ldn), f2(dn), ACTF.Ln, scale=1.0)
                STT(f2(ladv), f2(ldn), -2.0, f2(ldv), ALU.mult, ALU.add)
                nc.sync.dma_start(
                    _ap(out_d.ap(), [(128 * DH, GQ), (1, DH)],
                        offset_elems=g * GQ * 128 * DH, partitions=128),
                    f2(outv))
                nc.sync.dma_start(
                    _ap(lad_d.ap(), [(128 * DH, GQ), (1, DH)],
                        offset_elems=g * GQ * 128 * DH, partitions=128),
                    f2(ladv))

    nc.compile()
    _NC_CACHE[b_core] = nc
    return nc


def prep_in_maps(theta, x_conditioner, W1, b1, W2, b2, eta):
    theta = np.ascontiguousarray(np.asarray(theta, np.float32))
    x = np.asarray(x_conditioner, np.float32)
    W1 = np.ascontiguousarray(np.asarray(W1, np.float32))
    b1 = np.asarray(b1, np.float32)
    W2 = np.asarray(W2, np.float32)
    b2 = np.asarray(b2, np.float32)
    eta = float(np.asarray(eta).reshape(-1)[0])
    B = theta.shape[0]
    bc = B // NCORES

    # host prep: W2 cols permuted to [uw(256)|uh(256)|udx(264)], * eta;
    # b2 (and DERIV_SHIFT) ride row 256 (multiplied by an on-chip ones row)
    W2e = W2 * eta
    b2e = b2 * eta
    cols = np.arange(3 * K * DH).reshape(DH, 3, K)
    uw_cols = cols[:, 0, :].reshape(-1)
    uh_cols = cols[:, 1, :].reshape(-1)
    ud_cols = cols[:, 2, :]
    udx_cols = np.concatenate([ud_cols, ud_cols[:, :1]], 1).reshape(-1)
    w2p = np.empty((H + 1, 776), np.float32)
    w2p[:H, 0:256] = W2e[:, uw_cols]
    w2p[:H, 256:512] = W2e[:, uh_cols]
    w2p[:H, 512:776] = W2e[:, udx_cols]
    w2p[H, 0:256] = b2e[uw_cols]
    w2p[H, 256:512] = b2e[uh_cols]
    w2p[H, 512:776] = b2e[udx_cols] + DERIV_SHIFT
    b1r = np.ascontiguousarray(b1.reshape(2, 128).T)
    a_w = 1.0 - MIN_W * K
    basis = np.zeros((16, 256), np.float32)
    for d in range(DH):
        basis[d, 32 * d:32 * (d + 1)] = 1.0
        basis[8 + d, 32 * d:32 * (d + 1)] = -(np.arange(K) + 1) * MIN_W / a_w
    ident = np.eye(128, dtype=np.float32)

    in_maps = []
    for c in range(NCORES):
        sl = slice(c * bc, (c + 1) * bc)
        in_maps.append(dict(
            theta=theta[sl],
            xT=np.ascontiguousarray(x[sl].T),
            w1=W1, b1=b1r, w2=w2p, basis=basis, ident=ident))
    return in_maps


def kernel(theta, x_conditioner, W1, b1, W2, b2, eta):
    B = np.asarray(theta).shape[0]
    bc = B // NCORES
    nc = build_kernel(bc)
    in_maps = prep_in_maps(theta, x_conditioner, W1, b1, W2, b2, eta)
    res = run_bass_kernel_spmd(nc, in_maps, core_ids=list(range(NCORES)))
    outs = np.concatenate([r["outs"] for r in res.results], 0)
    lads = np.concatenate([r["lad"] for r in res.results], 0)
    return outs, lads


# revision 13
# speedup vs baseline: 1.0232x; 1.0232x over previous
"""Circular rational-quadratic spline flow on 8 Trainium2 cores.

Data-parallel over the batch (131072 rows -> 16384/core). Per core:
MLP on PE (f32r matmuls, relu via ACT), spline in a rows-on-partitions
layout. Bin search + all gathers are done with masked tensor_tensor_scan
tails: state=(data+state)*W, where complement masks W zero the state at
segment boundaries, so one scan over a [kill|e_1..e_32] x 8d stream
yields per-(row,d) tail sums whose differences give every gathered
quantity.

v2 changes vs baseline:
- all matmuls use float32r (1 cycle/col at N>=256 vs 4 for float32)
- the 7 masked tail scans are merged into 3 scan instructions
  (e-pair, f-pair, u-triple) via an extra leading AP dim; sub-stream
  boundaries are killed by the wx zero-pads, so one running state is safe
- work split across engines: f-pair scan, mj/r2, zw/zh reduces and part
  of the group-tail math run on GpSimd (Pool), freeing DVE
"""

import dataclasses
import numpy as np

import concourse.bacc as bacc
import concourse.mybir as mybir
import concourse.tile as tile
from concourse.bass_utils import run_bass_kernel_spmd

TWO_PI = 2.0 * np.pi
MIN_W = 1e-3
MIN_H = 1e-3
MIN_D = 1e-3
DERIV_SHIFT = float(np.log(np.e - 1.0))
K = 32
DH = 8
C = 64
H = 256
NCORES = 8

F32 = mybir.dt.float32
F32R = mybir.dt.float32r
ALU = mybir.AluOpType
AX = mybir.AxisListType
ACTF = mybir.ActivationFunctionType


def _ap(ap, dims, offset_elems=0, partitions=None):
    """AP with explicit free dims [(step, count), ...] in elements."""
    p = ap.ap[0]
    if partitions is not None:
        p = [p[0], partitions]
    aps = [p] + [[s, c] for (s, c) in dims]
    return dataclasses.replace(ap, ap=aps, offset=ap.offset + offset_elems)


def _scan_raw(nc, eng, out, data0, data1):
    """tensor_tensor_scan with multi-free-dim APs (bypasses 2D assert).
    state = (data0 + state) * data1 in AP stream order."""
    return eng.add_instruction(
        mybir.InstTensorScalarPtr(
            name=nc.get_next_instruction_name(),
            is_tensor_tensor_scan=True,
            is_scalar_tensor_tensor=True,
            op0=ALU.add,
            op1=ALU.mult,
            ins=[
                eng.lower_ap(data0),
                eng.lower_ap_or_imm(0.0),
                eng.lower_ap(data1),
            ],
            outs=[eng.lower_ap(out)],
        )
    )


def _r(ap_obj):
    """matmul input tiles are already float32r."""
    return ap_obj


_NC_CACHE = {}


def build_kernel(b_core):
    if b_core in _NC_CACHE:
        return _NC_CACHE[b_core]
    NQ = b_core // 128          # number of 128-row batch chunks
    GQ = min(64, NQ)            # chunks per tail group
    assert NQ % GQ == 0
    NG = NQ // GQ
    a_w = 1.0 - MIN_W * K
    a_h = 1.0 - MIN_H * K

    nc = bacc.Bacc("TRN2", debug=False)
    theta_d = nc.dram_tensor("theta", [b_core, DH], F32, kind="ExternalInput")
    xT_d = nc.dram_tensor("xT", [C, b_core], F32R, kind="ExternalInput")
    w1_d = nc.dram_tensor("w1", [C, H], F32R, kind="ExternalInput")
    b1_d = nc.dram_tensor("b1", [128, 2], F32, kind="ExternalInput")
    w2_d = nc.dram_tensor("w2", [H + 1, 776], F32R, kind="ExternalInput")
    basis_d = nc.dram_tensor("basis", [16, 256], F32R, kind="ExternalInput")
    ident_d = nc.dram_tensor("ident", [128, 128], F32, kind="ExternalInput")
    out_d = nc.dram_tensor("outs", [b_core, DH], F32, kind="ExternalOutput")
    lad_d = nc.dram_tensor("lad", [b_core, DH], F32, kind="ExternalOutput")

    with tile.TileContext(nc) as tc:
        with tc.tile_pool(name="const", bufs=1) as cpool, \
             tc.tile_pool(name="w2p", bufs=1) as wpool, \
             tc.tile_pool(name="mm1", bufs=2) as mpool, \
             tc.tile_pool(name="mm1ps", bufs=1, space="PSUM") as mm1ps, \
             tc.tile_pool(name="prps", bufs=2, space="PSUM") as prps, \
             tc.tile_pool(name="r2ps", bufs=2, space="PSUM") as r2pool, \
             tc.tile_pool(name="trps", bufs=1, space="PSUM") as trpool, \
             tc.tile_pool(name="chunk", bufs=8) as kpool, \
             tc.tile_pool(name="grp", bufs=2) as gpool, \
             tc.tile_pool(name="gtmp", bufs=1) as tpool:

            # ---------------- resident constants ----------------
            w1_t = cpool.tile([C, H], F32R)
            nc.sync.dma_start(w1_t[:, :], w1_d.ap())
            b1_t = cpool.tile([128, 2], F32)
            nc.sync.dma_start(b1_t[:, :], b1_d.ap())
            w2_t = wpool.tile([128, 2, 776], F32R)
            nc.sync.dma_start(
                w2_t[:, :, :],
                _ap(w2_d.ap(), [(776 * 128, 2), (1, 776)], partitions=128))
            b2row_t = cpool.tile([1, 776], F32R)
            nc.sync.dma_start(
                b2row_t[:, :],
                _ap(w2_d.ap(), [(1, 776)], offset_elems=256 * 776, partitions=1))
            ones_t = cpool.tile([1, 128], F32R)
            nc.vector.memset(ones_t[:, :].bitcast(F32), 1.0)
            basis_t = cpool.tile([16, 256], F32R)
            nc.sync.dma_start(basis_t[:, :], basis_d.ap())
            ident_t = cpool.tile([128, 128], F32)
            nc.sync.dma_start(ident_t[:, :], ident_d.ap())
            # per-partition bias constants for ACT-side affine tail ops
            cb_t = cpool.tile([128, 4], F32)
            nc.vector.memset(cb_t[:, 0:1], TWO_PI * MIN_W * K)
            nc.vector.memset(cb_t[:, 1:2], TWO_PI * MIN_W)
            nc.vector.memset(cb_t[:, 2:3], MIN_D)
            nc.vector.memset(cb_t[:, 3:4], 1.0)
            # carry-kill mask for the plain cumsum: 0 at stream col 33*d
            wc_t = cpool.tile([128, 264], F32)
            nc.vector.memset(wc_t[:, :], 1.0)
            nc.vector.memset(_ap(wc_t[:, :], [(33, DH), (1, 1)]), 0.0)
            # persistent ring buffers for ef / wx so constant pads are
            # written once instead of per chunk
            NB = 8
            # ef layout per buf: [pad(1) | e(256) | f(256) | pad(1)]; the e/f
            # streams then form one uniform 16-segment [(32,16),(1,33)] AP
            # (f seg d uses e's last element as its kill slot)
            efr_t = cpool.tile([128, NB, 514], F32)
            nc.vector.memset(_ap(efr_t[:, :, :], [(1, NB * 514)]), 1.0)
            wxr_t = cpool.tile([128, NB, 16, 36], F32)
            nc.vector.memset(_ap(wxr_t[:, :, :, :], [(1, NB * 16 * 36)]), 0.0)
            nc.vector.memset(
                _ap(wxr_t[:, :, :, :], [(36, NB * 16), (1, 1)], offset_elems=35),
                1.0)
            # 16-segment kill mask for the merged s/zh cumsum scan
            wc16_t = cpool.tile([128, 528], F32)
            nc.vector.memset(wc16_t[:, :], 1.0)
            nc.vector.memset(_ap(wc16_t[:, :], [(33, 16), (1, 1)]), 0.0)

            for g in range(NG):
                th_g = gpool.tile([128, GQ, DH], F32, tag="th")
                zw_g = gpool.tile([128, GQ, DH], F32, tag="zw")
                zh_g = gpool.tile([128, GQ, DH], F32, tag="zh")
                ix_g = gpool.tile([128, GQ, DH], F32, tag="ix")
                tl_g = gpool.tile([128, GQ, 7, DH], F32, tag="tl")
                nc.sync.dma_start(
                    th_g[:, :, :],
                    _ap(theta_d.ap(), [(128 * DH, GQ), (1, DH)],
                        offset_elems=g * GQ * 128 * DH, partitions=128))

                for qq in range(GQ):
                    q = g * GQ + qq
                    ql = q % 4
                    if ql == 0:
                        # -------- MM1 for the next 512 batch rows --------
                        xt_t = mpool.tile([C, 512], F32R, tag="xt")
                        nc.sync.dma_start(
                            xt_t[:, :],
                            _ap(xT_d.ap(), [(1, 512)], offset_elems=q * 128))
                        ht_t = mpool.tile([128, 2, 512], F32R, tag="ht")
                        for kh in range(2):
                            hps = mm1ps.tile([128, 512], F32, tag="hps")
                            nc.tensor.matmul(hps[:, :],
                                             _r(w1_t[:, 128 * kh:128 * (kh + 1)]),
                                             _r(xt_t[:, :]), start=True, stop=True)
                            nc.scalar.activation(ht_t[:, kh, :], hps[:, :],
                                                 ACTF.Relu,
                                                 bias=b1_t[:, kh:kh + 1],
                                                 scale=1.0)
                    # -------- MM2: params chunk -> PSUM [128, 776] --------
                    pps = prps.tile([128, 1024], F32, tag="pps")
                    for kh in range(2):
                        lhs = _r(ht_t[:, kh, 128 * ql:128 * (ql + 1)])
                        nc.tensor.matmul(pps[:, 0:512], lhs, _r(w2_t[:, kh, 0:512]),
                                         start=(kh == 0), stop=False)
                        nc.tensor.matmul(pps[:, 512:776], lhs,
                                         _r(w2_t[:, kh, 512:776]),
                                         start=(kh == 0), stop=False)
                    nc.tensor.matmul(pps[:, 0:512], _r(ones_t[:, :]),
                                     _r(b2row_t[:, 0:512]), start=False, stop=True)
                    nc.tensor.matmul(pps[:, 512:776], _r(ones_t[:, :]),
                                     _r(b2row_t[:, 512:776]), start=False, stop=True)

                    # -------- spline chunk --------
                    # EF[:, a, 1+32d+k] = exp(params)  (a=0: uw, a=1: uh)
                    ef_t = efr_t[:, q % NB]
                    nc.scalar.activation(
                        _ap(ef_t[:, :], [(256, 2), (1, 256)], offset_elems=1),
                        pps[:, 0:512], ACTF.Exp, scale=1.0)
                    # merged cumsum over 16 segments: S (e, segs 0-7) and
                    # cum-f (segs 8-15); Zw/Zh are the segment-end values
                    s_t = kpool.tile([128, 528], F32, tag="s")
                    _scan_raw(nc, nc.vector, s_t[:, :],
                              _ap(ef_t[:, :], [(32, 16), (1, 33)]),
                              wc16_t[:, :])
                    nc.gpsimd.tensor_copy(
                        zw_g[:, qq, :],
                        _ap(s_t[:, :], [(33, DH)], offset_elems=32))
                    nc.gpsimd.tensor_copy(
                        zh_g[:, qq, :],
                        _ap(s_t[:, :], [(33, DH)], offset_elems=264 + 32))
                    # smalls = [Tp | Zw] per row; R2 = basis.T-combo on PE:
                    # R2[row, 32d+k] = Tp[row,d] - (k+1)*MIN_W/a_w*Zw[row,d]
                    sm_t = kpool.tile([128, 16], F32, tag="sm")
                    nc.vector.scalar_tensor_tensor(
                        sm_t[:, 0:DH], th_g[:, qq, :], 1.0 / (TWO_PI * a_w),
                        zw_g[:, qq, :], ALU.mult, ALU.mult)
                    nc.gpsimd.tensor_copy(
                        sm_t[:, DH:16],
                        _ap(s_t[:, :], [(33, DH)], offset_elems=32))
                    smt_ps = trpool.tile([16, 128], F32, tag="smt")
                    nc.tensor.transpose(smt_ps[:, :], sm_t[:, :], ident_t[:, :])
                    smt_t = kpool.tile([16, 128], F32R, tag="smts")
                    nc.scalar.activation(smt_t[:, :], smt_ps[:, :],
                                         ACTF.Copy, scale=1.0)
                    r2_ps = r2pool.tile([128, 256], F32, tag="r2")
                    nc.tensor.matmul(r2_ps[:, :], smt_t[:, :], basis_t[:, :],
                                     start=True, stop=True)
                    # Wext[d, c] = W_{c-2} = [S_local > R2]; c in {0,1,2}->0, 35->1
                    wx_t = wxr_t[:, q % NB]
                    nc.vector.tensor_tensor(
                        _ap(wx_t[:, :, :], [(36, DH), (1, K)], offset_elems=3),
                        _ap(s_t[:, :], [(33, DH), (1, K)], offset_elems=1),
                        _ap(r2_ps[:, :], [(K, DH), (1, K)]), ALU.is_gt)
                    nc.gpsimd.tensor_copy(
                        _ap(wx_t[:, :, :], [(36, DH), (1, K)],
                            offset_elems=36 * DH + 3),
                        _ap(wx_t[:, :, :], [(36, DH), (1, K)], offset_elems=3))
                    # idx = 32 - sum(W)
                    nc.vector.tensor_reduce(
                        ix_g[:, qq, :],
                        _ap(wx_t[:, :, :], [(36, DH), (1, K)], offset_elems=3),
                        axis=AX.X, op=ALU.add)
                    # ud params psum -> SBUF (GPSIMD cannot read PSUM)
                    ud_t = kpool.tile([128, 265], F32, tag="ud")
                    nc.scalar.activation(ud_t[:, :], pps[:, 511:776],
                                         ACTF.Copy, scale=1.0)
                    # -------- 7 masked tail scans --------
                    # out APs overlap with stride 0 along the stream, so each
                    # d-segment's 33/34 running values land on one address and
                    # the final value (the masked tail) survives -> tails are
                    # written directly into the group tile, no scratch/copies
                    # merged e+f tails (16 segs -> tl slots t_i, t_i+1)
                    for (t_i, sh) in ((0, 0), (2, 1)):
                        _scan_raw(
                            nc, nc.vector,
                            _ap(tl_g[:, :, :, :], [(1, 16), (0, 33)],
                                offset_elems=(qq * 7 + t_i) * DH),
                            _ap(ef_t[:, :], [(32, 16), (1, 33)]),
                            _ap(wx_t[:, :, :], [(36, 16), (1, 33)],
                                offset_elems=2 - sh))
                    for (t_i, sh) in ((4, 0), (5, 1), (6, 2)):
                        _scan_raw(
                            nc, nc.vector,
                            _ap(tl_g[:, :, :, :], [(1, DH), (0, 34)],
                                offset_elems=(qq * 7 + t_i) * DH),
                            _ap(ud_t[:, :], [(33, DH), (1, 34)]),
                            _ap(wx_t[:, :, :], [(36, DH), (1, 34)],
                                offset_elems=2 - sh))

                # ============ group tail: per-row rational quadratic ============
                def gt(tag):
                    return tpool.tile([128, GQ, DH], F32, tag=tag, name=tag)

                def f2(t):
                    return t[:, :, :]

                TT = nc.vector.tensor_tensor
                STT = nc.vector.scalar_tensor_tensor
                PTT = nc.vector.tensor_tensor
                PSTT = nc.vector.scalar_tensor_tensor
                rzw = gt("rzw"); rzh = gt("rzh")
                nc.vector.reciprocal(f2(rzw), f2(zw_g))
                nc.vector.reciprocal(f2(rzh), f2(zh_g))
                sm0 = gt("sm0"); sm1 = gt("sm1"); hm0 = gt("hm0"); hm1 = gt("hm1")
                PTT(f2(sm0), f2(zw_g), tl_g[:, :, 0, :], ALU.subtract)
                PTT(f2(sm1), f2(zw_g), tl_g[:, :, 2, :], ALU.subtract)
                PTT(f2(hm0), f2(zh_g), tl_g[:, :, 1, :], ALU.subtract)
                PTT(f2(hm1), f2(zh_g), tl_g[:, :, 3, :], ALU.subtract)
                u1 = gt("u1"); u2 = gt("u2")
                PTT(f2(u1), tl_g[:, :, 4, :], tl_g[:, :, 5, :], ALU.subtract)
                PTT(f2(u2), tl_g[:, :, 5, :], tl_g[:, :, 6, :], ALU.subtract)
                icw = gt("icw"); inw = gt("inw"); ich = gt("ich"); inh = gt("inh")
                tmp = gt("tmp"); tmp2 = gt("tmp2")
                STT(f2(tmp), f2(sm0), TWO_PI * a_w, f2(rzw), ALU.mult, ALU.mult)
                STT(f2(icw), f2(ix_g), -TWO_PI * MIN_W, f2(tmp), ALU.mult, ALU.add)
                nc.scalar.activation(f2(icw), f2(icw), ACTF.Identity, bias=cb_t[:, 0:1])
                PTT(f2(tmp2), f2(sm1), f2(sm0), ALU.subtract)
                STT(f2(tmp), f2(tmp2), TWO_PI * a_w, f2(rzw), ALU.mult, ALU.mult)
                nc.scalar.activation(f2(inw), f2(tmp), ACTF.Identity, bias=cb_t[:, 1:2])
                PSTT(f2(tmp), f2(hm0), TWO_PI * a_h, f2(rzh), ALU.mult, ALU.mult)
                PSTT(f2(ich), f2(ix_g), -TWO_PI * MIN_H, f2(tmp), ALU.mult, ALU.add)
                nc.scalar.activation(f2(ich), f2(ich), ACTF.Identity, bias=cb_t[:, 0:1])
                PTT(f2(tmp2), f2(hm1), f2(hm0), ALU.subtract)
                PSTT(f2(tmp), f2(tmp2), TWO_PI * a_h, f2(rzh), ALU.mult, ALU.mult)
                nc.scalar.activation(f2(inh), f2(tmp), ACTF.Identity, bias=cb_t[:, 1:2])
                # d0/d1 = MIN_D + ln(1 + exp(u))
                e1 = gt("e1"); e2 = gt("e2"); dd0 = gt("dd0"); dd1 = gt("dd1")
                nc.scalar.activation(f2(e1), f2(u1), ACTF.Exp, scale=1.0)
                nc.scalar.activation(f2(e2), f2(u2), ACTF.Exp, scale=1.0)
                nc.scalar.activation(f2(dd0), f2(e1), ACTF.Ln, bias=cb_t[:, 3:4],
                                     scale=1.0)
                nc.scalar.activation(f2(dd1), f2(e2), ACTF.Ln, bias=cb_t[:, 3:4],
                                     scale=1.0)
                nc.scalar.activation(f2(dd0), f2(dd0), ACTF.Identity, bias=cb_t[:, 2:3])
                nc.scalar.activation(f2(dd1), f2(dd1), ACTF.Identity, bias=cb_t[:, 2:3])
                rw = gt("rw"); tt_ = gt("tt"); t1 = gt("t1")
                nc.vector.reciprocal(f2(rw), f2(inw))
                TT(f2(tmp), f2(th_g), f2(icw), ALU.subtract)
                TT(f2(tt_), f2(tmp), f2(rw), ALU.mult)
                nc.scalar.activation(f2(tmp), f2(tt_), ACTF.Identity,
                                     bias=cb_t[:, 3:4], scale=-1.0)   # 1 - t
                TT(f2(t1), f2(tt_), f2(tmp), ALU.mult)
                dl = gt("dl"); t2 = gt("t2"); omt2 = gt("omt2")
                PTT(f2(dl), f2(inh), f2(rw), ALU.mult)
                TT(f2(t2), f2(tt_), f2(tt_), ALU.mult)
                PTT(f2(omt2), f2(tmp), f2(tmp), ALU.mult)
                nm = gt("nm"); dn = gt("dn")
                TT(f2(tmp2), f2(dl), f2(t2), ALU.mult)
                TT(f2(nm), f2(dd0), f2(t1), ALU.mult)
                TT(f2(nm), f2(nm), f2(tmp2), ALU.add)
                TT(f2(nm), f2(nm), f2(inh), ALU.mult)
                PTT(f2(dn), f2(dd0), f2(dd1), ALU.add)
                STT(f2(dn), f2(dl), -2.0, f2(dn), ALU.mult, ALU.add)
                TT(f2(dn), f2(dn), f2(t1), ALU.mult)
                TT(f2(dn), f2(dn), f2(dl), ALU.add)
                rdn = gt("rdn"); outv = gt("outv")
                nc.vector.reciprocal(f2(rdn), f2(dn))
                TT(f2(outv), f2(nm), f2(rdn), ALU.mult)
                TT(f2(outv), f2(outv), f2(ich), ALU.add)
                dv = gt("dv")
                PTT(f2(dv), f2(dd1), f2(t2), ALU.mult)
                PSTT(f2(tmp2), f2(dl), 2.0, f2(t1), ALU.mult, ALU.mult)
                PTT(f2(dv), f2(dv), f2(tmp2), ALU.add)
                PTT(f2(tmp2), f2(dd0), f2(omt2), ALU.mult)
                PTT(f2(dv), f2(dv), f2(tmp2), ALU.add)
                PTT(f2(tmp2), f2(dl), f2(dl), ALU.mult)
                PTT(f2(dv), f2(dv), f2(tmp2), ALU.mult)
                ldv = gt("ldv"); ldn = gt("ldn"); ladv = gt("ladv")
                nc.scalar.activation(f2(ldv), f2(dv), ACTF.Ln, scale=1.0)
                nc.scalar.activation(f2(ldn), f2(dn), ACTF.Ln, scale=1.0)
                STT(f2(ladv), f2(ldn), -2.0, f2(ldv), ALU.mult, ALU.add)
                nc.sync.dma_start(
                    _ap(out_d.ap(), [(128 * DH, GQ), (1, DH)],
                        offset_elems=g * GQ * 128 * DH, partitions=128),
                    f2(outv))
                nc.sync.dma_start(
                    _ap(lad_d.ap(), [(128 * DH, GQ), (1, DH)],
                        offset_elems=g * GQ * 128 * DH, partitions=128),
                    f2(ladv))

    nc.compile()
    _NC_CACHE[b_core] = nc
    return nc


def prep_in_maps(theta, x_conditioner, W1, b1, W2, b2, eta):
    theta = np.ascontiguousarray(np.asarray(theta, np.float32))
    x = np.asarray(x_conditioner, np.float32)
    W1 = np.ascontiguousarray(np.asarray(W1, np.float32))
    b1 = np.asarray(b1, np.float32)
    W2 = np.asarray(W2, np.float32)
    b2 = np.asarray(b2, np.float32)
    eta = float(np.asarray(eta).reshape(-1)[0])
    B = theta.shape[0]
    bc = B // NCORES

    # host prep: W2 cols permuted to [uw(256)|uh(256)|udx(264)], * eta;
    # b2 (and DERIV_SHIFT) ride row 256 (multiplied by an on-chip ones row)
    W2e = W2 * eta
    b2e = b2 * eta
    cols = np.arange(3 * K * DH).reshape(DH, 3, K)
    uw_cols = cols[:, 0, :].reshape(-1)
    uh_cols = cols[:, 1, :].reshape(-1)
    ud_cols = cols[:, 2, :]
    udx_cols = np.concatenate([ud_cols, ud_cols[:, :1]], 1).reshape(-1)
    w2p = np.empty((H + 1, 776), np.float32)
    w2p[:H, 0:256] = W2e[:, uw_cols]
    w2p[:H, 256:512] = W2e[:, uh_cols]
    w2p[:H, 512:776] = W2e[:, udx_cols]
    w2p[H, 0:256] = b2e[uw_cols]
    w2p[H, 256:512] = b2e[uh_cols]
    w2p[H, 512:776] = b2e[udx_cols] + DERIV_SHIFT
    b1r = np.ascontiguousarray(b1.reshape(2, 128).T)
    a_w = 1.0 - MIN_W * K
    basis = np.zeros((16, 256), np.float32)
    for d in range(DH):
        basis[d, 32 * d:32 * (d + 1)] = 1.0
        basis[8 + d, 32 * d:32 * (d + 1)] = -(np.arange(K) + 1) * MIN_W / a_w
    ident = np.eye(128, dtype=np.float32)

    in_maps = []
    for c in range(NCORES):
        sl = slice(c * bc, (c + 1) * bc)
        in_maps.append(dict(
            theta=theta[sl],
            xT=np.ascontiguousarray(x[sl].T),
            w1=W1, b1=b1r, w2=w2p, basis=basis, ident=ident))
    return in_maps


def kernel(theta, x_conditioner, W1, b1, W2, b2, eta):
    B = np.asarray(theta).shape[0]
    bc = B // NCORES
    nc = build_kernel(bc)
    in_maps = prep_in_maps(theta, x_conditioner, W1, b1, W2, b2, eta)
    res = run_bass_kernel_spmd(nc, in_maps, core_ids=list(range(NCORES)))
    outs = np.concatenate([r["outs"] for r in res.results], 0)
    lads = np.concatenate([r["lad"] for r in res.results], 0)
    return outs, lads


# revision 14
# speedup vs baseline: 1.0256x; 1.0024x over previous
"""Circular rational-quadratic spline flow on 8 Trainium2 cores.

Data-parallel over the batch (131072 rows -> 16384/core). Per core:
MLP on PE (f32r matmuls, relu via ACT), spline in a rows-on-partitions
layout. Bin search + all gathers are done with masked tensor_tensor_scan
tails: state=(data+state)*W, where complement masks W zero the state at
segment boundaries, so one scan over a [kill|e_1..e_32] x 8d stream
yields per-(row,d) tail sums whose differences give every gathered
quantity.

v2 changes vs baseline:
- all matmuls use float32r (1 cycle/col at N>=256 vs 4 for float32)
- the 7 masked tail scans are merged into 3 scan instructions
  (e-pair, f-pair, u-triple) via an extra leading AP dim; sub-stream
  boundaries are killed by the wx zero-pads, so one running state is safe
- work split across engines: f-pair scan, mj/r2, zw/zh reduces and part
  of the group-tail math run on GpSimd (Pool), freeing DVE
"""

import dataclasses
import numpy as np

import concourse.bacc as bacc
import concourse.mybir as mybir
import concourse.tile as tile
from concourse.bass_utils import run_bass_kernel_spmd

TWO_PI = 2.0 * np.pi
MIN_W = 1e-3
MIN_H = 1e-3
MIN_D = 1e-3
DERIV_SHIFT = float(np.log(np.e - 1.0))
K = 32
DH = 8
C = 64
H = 256
NCORES = 8

F32 = mybir.dt.float32
F32R = mybir.dt.float32r
ALU = mybir.AluOpType
AX = mybir.AxisListType
ACTF = mybir.ActivationFunctionType


def _ap(ap, dims, offset_elems=0, partitions=None):
    """AP with explicit free dims [(step, count), ...] in elements."""
    p = ap.ap[0]
    if partitions is not None:
        p = [p[0], partitions]
    aps = [p] + [[s, c] for (s, c) in dims]
    return dataclasses.replace(ap, ap=aps, offset=ap.offset + offset_elems)


def _scan_raw(nc, eng, out, data0, data1):
    """tensor_tensor_scan with multi-free-dim APs (bypasses 2D assert).
    state = (data0 + state) * data1 in AP stream order."""
    return eng.add_instruction(
        mybir.InstTensorScalarPtr(
            name=nc.get_next_instruction_name(),
            is_tensor_tensor_scan=True,
            is_scalar_tensor_tensor=True,
            op0=ALU.add,
            op1=ALU.mult,
            ins=[
                eng.lower_ap(data0),
                eng.lower_ap_or_imm(0.0),
                eng.lower_ap(data1),
            ],
            outs=[eng.lower_ap(out)],
        )
    )


def _r(ap_obj):
    """matmul input tiles are already float32r."""
    return ap_obj


_NC_CACHE = {}


def build_kernel(b_core):
    if b_core in _NC_CACHE:
        return _NC_CACHE[b_core]
    NQ = b_core // 128          # number of 128-row batch chunks
    GQ = min(64, NQ)            # chunks per tail group
    assert NQ % GQ == 0
    NG = NQ // GQ
    a_w = 1.0 - MIN_W * K
    a_h = 1.0 - MIN_H * K

    nc = bacc.Bacc("TRN2", debug=False)
    theta_d = nc.dram_tensor("theta", [b_core, DH], F32, kind="ExternalInput")
    xT_d = nc.dram_tensor("xT", [C, b_core], F32R, kind="ExternalInput")
    w1_d = nc.dram_tensor("w1", [C, H], F32R, kind="ExternalInput")
    b1_d = nc.dram_tensor("b1", [128, 2], F32, kind="ExternalInput")
    w2_d = nc.dram_tensor("w2", [H + 1, 776], F32R, kind="ExternalInput")
    basis_d = nc.dram_tensor("basis", [16, 256], F32R, kind="ExternalInput")
    ident_d = nc.dram_tensor("ident", [128, 128], F32, kind="ExternalInput")
    out_d = nc.dram_tensor("outs", [b_core, DH], F32, kind="ExternalOutput")
    lad_d = nc.dram_tensor("lad", [b_core, DH], F32, kind="ExternalOutput")

    with tile.TileContext(nc) as tc:
        with tc.tile_pool(name="const", bufs=1) as cpool, \
             tc.tile_pool(name="w2p", bufs=1) as wpool, \
             tc.tile_pool(name="mm1", bufs=2) as mpool, \
             tc.tile_pool(name="mm1ps", bufs=2, space="PSUM") as mm1ps, \
             tc.tile_pool(name="prps", bufs=2, space="PSUM") as prps, \
             tc.tile_pool(name="r2ps", bufs=1, space="PSUM") as r2pool, \
             tc.tile_pool(name="trps", bufs=1, space="PSUM") as trpool, \
             tc.tile_pool(name="chunk", bufs=8) as kpool, \
             tc.tile_pool(name="grp", bufs=2) as gpool, \
             tc.tile_pool(name="gtmp", bufs=1) as tpool:

            # ---------------- resident constants ----------------
            w1_t = cpool.tile([C, H], F32R)
            nc.sync.dma_start(w1_t[:, :], w1_d.ap())
            b1_t = cpool.tile([128, 2], F32)
            nc.sync.dma_start(b1_t[:, :], b1_d.ap())
            w2_t = wpool.tile([128, 2, 776], F32R)
            nc.sync.dma_start(
                w2_t[:, :, :],
                _ap(w2_d.ap(), [(776 * 128, 2), (1, 776)], partitions=128))
            b2row_t = cpool.tile([1, 776], F32R)
            nc.sync.dma_start(
                b2row_t[:, :],
                _ap(w2_d.ap(), [(1, 776)], offset_elems=256 * 776, partitions=1))
            ones_t = cpool.tile([1, 128], F32R)
            nc.vector.memset(ones_t[:, :].bitcast(F32), 1.0)
            basis_t = cpool.tile([16, 256], F32R)
            nc.sync.dma_start(basis_t[:, :], basis_d.ap())
            ident_t = cpool.tile([128, 128], F32)
            nc.sync.dma_start(ident_t[:, :], ident_d.ap())
            # per-partition bias constants for ACT-side affine tail ops
            cb_t = cpool.tile([128, 4], F32)
            nc.vector.memset(cb_t[:, 0:1], TWO_PI * MIN_W * K)
            nc.vector.memset(cb_t[:, 1:2], TWO_PI * MIN_W)
            nc.vector.memset(cb_t[:, 2:3], MIN_D)
            nc.vector.memset(cb_t[:, 3:4], 1.0)
            # carry-kill mask for the plain cumsum: 0 at stream col 33*d
            wc_t = cpool.tile([128, 264], F32)
            nc.vector.memset(wc_t[:, :], 1.0)
            nc.vector.memset(_ap(wc_t[:, :], [(33, DH), (1, 1)]), 0.0)
            # persistent ring buffers for ef / wx so constant pads are
            # written once instead of per chunk
            NB = 8
            # ef layout per buf: [pad(1) | e(256) | f(256) | pad(1)]; the e/f
            # streams then form one uniform 16-segment [(32,16),(1,33)] AP
            # (f seg d uses e's last element as its kill slot)
            efr_t = cpool.tile([128, NB, 514], F32)
            nc.vector.memset(_ap(efr_t[:, :, :], [(1, NB * 514)]), 1.0)
            wxr_t = cpool.tile([128, NB, 16, 36], F32)
            nc.vector.memset(_ap(wxr_t[:, :, :, :], [(1, NB * 16 * 36)]), 0.0)
            nc.vector.memset(
                _ap(wxr_t[:, :, :, :], [(36, NB * 16), (1, 1)], offset_elems=35),
                1.0)
            # 16-segment kill mask for the merged s/zh cumsum scan
            wc16_t = cpool.tile([128, 528], F32)
            nc.vector.memset(wc16_t[:, :], 1.0)
            nc.vector.memset(_ap(wc16_t[:, :], [(33, 16), (1, 1)]), 0.0)

            for g in range(NG):
                th_g = gpool.tile([128, GQ, DH], F32, tag="th")
                zw_g = gpool.tile([128, GQ, DH], F32, tag="zw")
                zh_g = gpool.tile([128, GQ, DH], F32, tag="zh")
                ix_g = gpool.tile([128, GQ, DH], F32, tag="ix")
                tl_g = gpool.tile([128, GQ, 7, DH], F32, tag="tl")
                nc.sync.dma_start(
                    th_g[:, :, :],
                    _ap(theta_d.ap(), [(128 * DH, GQ), (1, DH)],
                        offset_elems=g * GQ * 128 * DH, partitions=128))

                for qq in range(GQ):
                    q = g * GQ + qq
                    ql = q % 4
                    if ql == 0:
                        # -------- MM1 for the next 512 batch rows --------
                        xt_t = mpool.tile([C, 512], F32R, tag="xt")
                        nc.sync.dma_start(
                            xt_t[:, :],
                            _ap(xT_d.ap(), [(1, 512)], offset_elems=q * 128))
                        ht_t = mpool.tile([128, 2, 512], F32R, tag="ht")
                        for kh in range(2):
                            hps = mm1ps.tile([128, 512], F32, tag="hps")
                            nc.tensor.matmul(hps[:, :],
                                             _r(w1_t[:, 128 * kh:128 * (kh + 1)]),
                                             _r(xt_t[:, :]), start=True, stop=True)
                            nc.scalar.activation(ht_t[:, kh, :], hps[:, :],
                                                 ACTF.Relu,
                                                 bias=b1_t[:, kh:kh + 1],
                                                 scale=1.0)
                    # -------- MM2: params chunk -> PSUM [128, 776] --------
                    pps = prps.tile([128, 1024], F32, tag="pps")
                    for kh in range(2):
                        lhs = _r(ht_t[:, kh, 128 * ql:128 * (ql + 1)])
                        nc.tensor.matmul(pps[:, 0:512], lhs, _r(w2_t[:, kh, 0:512]),
                                         start=(kh == 0), stop=False)
                        nc.tensor.matmul(pps[:, 512:776], lhs,
                                         _r(w2_t[:, kh, 512:776]),
                                         start=(kh == 0), stop=False)
                    nc.tensor.matmul(pps[:, 0:512], _r(ones_t[:, :]),
                                     _r(b2row_t[:, 0:512]), start=False, stop=True)
                    nc.tensor.matmul(pps[:, 512:776], _r(ones_t[:, :]),
                                     _r(b2row_t[:, 512:776]), start=False, stop=True)

                    # -------- spline chunk --------
                    # EF[:, a, 1+32d+k] = exp(params)  (a=0: uw, a=1: uh)
                    ef_t = efr_t[:, q % NB]
                    nc.scalar.activation(
                        _ap(ef_t[:, :], [(256, 2), (1, 256)], offset_elems=1),
                        pps[:, 0:512], ACTF.Exp, scale=1.0)
                    # merged cumsum over 16 segments: S (e, segs 0-7) and
                    # cum-f (segs 8-15); Zw/Zh are the segment-end values
                    s_t = kpool.tile([128, 528], F32, tag="s")
                    _scan_raw(nc, nc.vector, s_t[:, :],
                              _ap(ef_t[:, :], [(32, 16), (1, 33)]),
                              wc16_t[:, :])
                    nc.gpsimd.tensor_copy(
                        zw_g[:, qq, :],
                        _ap(s_t[:, :], [(33, DH)], offset_elems=32))
                    nc.gpsimd.tensor_copy(
                        zh_g[:, qq, :],
                        _ap(s_t[:, :], [(33, DH)], offset_elems=264 + 32))
                    # smalls = [Tp | Zw] per row; R2 = basis.T-combo on PE:
                    # R2[row, 32d+k] = Tp[row,d] - (k+1)*MIN_W/a_w*Zw[row,d]
                    sm_t = kpool.tile([128, 16], F32, tag="sm")
                    nc.vector.scalar_tensor_tensor(
                        sm_t[:, 0:DH], th_g[:, qq, :], 1.0 / (TWO_PI * a_w),
                        zw_g[:, qq, :], ALU.mult, ALU.mult)
                    nc.gpsimd.tensor_copy(
                        sm_t[:, DH:16],
                        _ap(s_t[:, :], [(33, DH)], offset_elems=32))
                    smt_ps = trpool.tile([16, 128], F32, tag="smt")
                    nc.tensor.transpose(smt_ps[:, :], sm_t[:, :], ident_t[:, :])
                    smt_t = kpool.tile([16, 128], F32R, tag="smts")
                    nc.scalar.activation(smt_t[:, :], smt_ps[:, :],
                                         ACTF.Copy, scale=1.0)
                    r2_ps = r2pool.tile([128, 256], F32, tag="r2")
                    nc.tensor.matmul(r2_ps[:, :], smt_t[:, :], basis_t[:, :],
                                     start=True, stop=True)
                    # Wext[d, c] = W_{c-2} = [S_local > R2]; c in {0,1,2}->0, 35->1
                    wx_t = wxr_t[:, q % NB]
                    nc.vector.tensor_tensor(
                        _ap(wx_t[:, :, :], [(36, DH), (1, K)], offset_elems=3),
                        _ap(s_t[:, :], [(33, DH), (1, K)], offset_elems=1),
                        _ap(r2_ps[:, :], [(K, DH), (1, K)]), ALU.is_gt)
                    nc.gpsimd.tensor_copy(
                        _ap(wx_t[:, :, :], [(36, DH), (1, K)],
                            offset_elems=36 * DH + 3),
                        _ap(wx_t[:, :, :], [(36, DH), (1, K)], offset_elems=3))
                    # idx = 32 - sum(W)
                    nc.vector.tensor_reduce(
                        ix_g[:, qq, :],
                        _ap(wx_t[:, :, :], [(36, DH), (1, K)], offset_elems=3),
                        axis=AX.X, op=ALU.add)
                    # ud params psum -> SBUF (GPSIMD cannot read PSUM)
                    ud_t = kpool.tile([128, 265], F32, tag="ud")
                    nc.scalar.activation(ud_t[:, :], pps[:, 511:776],
                                         ACTF.Copy, scale=1.0)
                    # -------- 7 masked tail scans --------
                    # out APs overlap with stride 0 along the stream, so each
                    # d-segment's 33/34 running values land on one address and
                    # the final value (the masked tail) survives -> tails are
                    # written directly into the group tile, no scratch/copies
                    # merged e+f tails (16 segs -> tl slots t_i, t_i+1)
                    for (t_i, sh) in ((0, 0), (2, 1)):
                        _scan_raw(
                            nc, nc.vector,
                            _ap(tl_g[:, :, :, :], [(1, 16), (0, 33)],
                                offset_elems=(qq * 7 + t_i) * DH),
                            _ap(ef_t[:, :], [(32, 16), (1, 33)]),
                            _ap(wx_t[:, :, :], [(36, 16), (1, 33)],
                                offset_elems=2 - sh))
                    for (t_i, sh) in ((4, 0), (5, 1), (6, 2)):
                        _scan_raw(
                            nc, nc.vector,
                            _ap(tl_g[:, :, :, :], [(1, DH), (0, 34)],
                                offset_elems=(qq * 7 + t_i) * DH),
                            _ap(ud_t[:, :], [(33, DH), (1, 34)]),
                            _ap(wx_t[:, :, :], [(36, DH), (1, 34)],
                                offset_elems=2 - sh))

                # ============ group tail: per-row rational quadratic ============
                def gt(tag):
                    return tpool.tile([128, GQ, DH], F32, tag=tag, name=tag)

                def f2(t):
                    return t[:, :, :]

                TT = nc.vector.tensor_tensor
                STT = nc.vector.scalar_tensor_tensor
                PTT = nc.vector.tensor_tensor
                PSTT = nc.vector.scalar_tensor_tensor
                rzw = gt("rzw"); rzh = gt("rzh")
                nc.vector.reciprocal(f2(rzw), f2(zw_g))
                nc.vector.reciprocal(f2(rzh), f2(zh_g))
                sm0 = gt("sm0"); sm1 = gt("sm1"); hm0 = gt("hm0"); hm1 = gt("hm1")
                PTT(f2(sm0), f2(zw_g), tl_g[:, :, 0, :], ALU.subtract)
                PTT(f2(sm1), f2(zw_g), tl_g[:, :, 2, :], ALU.subtract)
                PTT(f2(hm0), f2(zh_g), tl_g[:, :, 1, :], ALU.subtract)
                PTT(f2(hm1), f2(zh_g), tl_g[:, :, 3, :], ALU.subtract)
                u1 = gt("u1"); u2 = gt("u2")
                PTT(f2(u1), tl_g[:, :, 4, :], tl_g[:, :, 5, :], ALU.subtract)
                PTT(f2(u2), tl_g[:, :, 5, :], tl_g[:, :, 6, :], ALU.subtract)
                icw = gt("icw"); inw = gt("inw"); ich = gt("ich"); inh = gt("inh")
                tmp = gt("tmp"); tmp2 = gt("tmp2")
                STT(f2(tmp), f2(sm0), TWO_PI * a_w, f2(rzw), ALU.mult, ALU.mult)
                STT(f2(icw), f2(ix_g), -TWO_PI * MIN_W, f2(tmp), ALU.mult, ALU.add)
                nc.scalar.activation(f2(icw), f2(icw), ACTF.Identity, bias=cb_t[:, 0:1])
                PTT(f2(tmp2), f2(sm1), f2(sm0), ALU.subtract)
                STT(f2(tmp), f2(tmp2), TWO_PI * a_w, f2(rzw), ALU.mult, ALU.mult)
                nc.scalar.activation(f2(inw), f2(tmp), ACTF.Identity, bias=cb_t[:, 1:2])
                PSTT(f2(tmp), f2(hm0), TWO_PI * a_h, f2(rzh), ALU.mult, ALU.mult)
                PSTT(f2(ich), f2(ix_g), -TWO_PI * MIN_H, f2(tmp), ALU.mult, ALU.add)
                nc.scalar.activation(f2(ich), f2(ich), ACTF.Identity, bias=cb_t[:, 0:1])
                PTT(f2(tmp2), f2(hm1), f2(hm0), ALU.subtract)
                PSTT(f2(tmp), f2(tmp2), TWO_PI * a_h, f2(rzh), ALU.mult, ALU.mult)
                nc.scalar.activation(f2(inh), f2(tmp), ACTF.Identity, bias=cb_t[:, 1:2])
                # d0/d1 = MIN_D + ln(1 + exp(u))
                e1 = gt("e1"); e2 = gt("e2"); dd0 = gt("dd0"); dd1 = gt("dd1")
                nc.scalar.activation(f2(e1), f2(u1), ACTF.Exp, scale=1.0)
                nc.scalar.activation(f2(e2), f2(u2), ACTF.Exp, scale=1.0)
                nc.scalar.activation(f2(dd0), f2(e1), ACTF.Ln, bias=cb_t[:, 3:4],
                                     scale=1.0)
                nc.scalar.activation(f2(dd1), f2(e2), ACTF.Ln, bias=cb_t[:, 3:4],
                                     scale=1.0)
                nc.scalar.activation(f2(dd0), f2(dd0), ACTF.Identity, bias=cb_t[:, 2:3])
                nc.scalar.activation(f2(dd1), f2(dd1), ACTF.Identity, bias=cb_t[:, 2:3])
                rw = gt("rw"); tt_ = gt("tt"); t1 = gt("t1")
                nc.vector.reciprocal(f2(rw), f2(inw))
                TT(f2(tmp), f2(th_g), f2(icw), ALU.subtract)
                TT(f2(tt_), f2(tmp), f2(rw), ALU.mult)
                nc.scalar.activation(f2(tmp), f2(tt_), ACTF.Identity,
                                     bias=cb_t[:, 3:4], scale=-1.0)   # 1 - t
                TT(f2(t1), f2(tt_), f2(tmp), ALU.mult)
                dl = gt("dl"); t2 = gt("t2"); omt2 = gt("omt2")
                PTT(f2(dl), f2(inh), f2(rw), ALU.mult)
                TT(f2(t2), f2(tt_), f2(tt_), ALU.mult)
                PTT(f2(omt2), f2(tmp), f2(tmp), ALU.mult)
                nm = gt("nm"); dn = gt("dn")
                TT(f2(tmp2), f2(dl), f2(t2), ALU.mult)
                TT(f2(nm), f2(dd0), f2(t1), ALU.mult)
                TT(f2(nm), f2(nm), f2(tmp2), ALU.add)
                TT(f2(nm), f2(nm), f2(inh), ALU.mult)
                PTT(f2(dn), f2(dd0), f2(dd1), ALU.add)
                STT(f2(dn), f2(dl), -2.0, f2(dn), ALU.mult, ALU.add)
                TT(f2(dn), f2(dn), f2(t1), ALU.mult)
                TT(f2(dn), f2(dn), f2(dl), ALU.add)
                rdn = gt("rdn"); outv = gt("outv")
                nc.vector.reciprocal(f2(rdn), f2(dn))
                TT(f2(outv), f2(nm), f2(rdn), ALU.mult)
                TT(f2(outv), f2(outv), f2(ich), ALU.add)
                dv = gt("dv")
                PTT(f2(dv), f2(dd1), f2(t2), ALU.mult)
                PSTT(f2(tmp2), f2(dl), 2.0, f2(t1), ALU.mult, ALU.mult)
                PTT(f2(dv), f2(dv), f2(tmp2), ALU.add)
                PTT(f2(tmp2), f2(dd0), f2(omt2), ALU.mult)
                PTT(f2(dv), f2(dv), f2(tmp2), ALU.add)
                PTT(f2(tmp2), f2(dl), f2(dl), ALU.mult)
                PTT(f2(dv), f2(dv), f2(tmp2), ALU.mult)
                ldv = gt("ldv"); ldn = gt("ldn"); ladv = gt("ladv")
                nc.scalar.activation(f2(ldv), f2(dv), ACTF.Ln, scale=1.0)
                nc.scalar.activation(f2(ldn), f2(dn), ACTF.Ln, scale=1.0)
                STT(f2(ladv), f2(ldn), -2.0, f2(ldv), ALU.mult, ALU.add)
                nc.sync.dma_start(
                    _ap(out_d.ap(), [(128 * DH, GQ), (1, DH)],
                        offset_elems=g * GQ * 128 * DH, partitions=128),
                    f2(outv))
                nc.sync.dma_start(
                    _ap(lad_d.ap(), [(128 * DH, GQ), (1, DH)],
                        offset_elems=g * GQ * 128 * DH, partitions=128),
                    f2(ladv))

    nc.compile()
    _NC_CACHE[b_core] = nc
    return nc


def prep_in_maps(theta, x_conditioner, W1, b1, W2, b2, eta):
    theta = np.ascontiguousarray(np.asarray(theta, np.float32))
    x = np.asarray(x_conditioner, np.float32)
    W1 = np.ascontiguousarray(np.asarray(W1, np.float32))
    b1 = np.asarray(b1, np.float32)
    W2 = np.asarray(W2, np.float32)
    b2 = np.asarray(b2, np.float32)
    eta = float(np.asarray(eta).reshape(-1)[0])
    B = theta.shape[0]
    bc = B // NCORES

    # host prep: W2 cols permuted to [uw(256)|uh(256)|udx(264)], * eta;
    # b2 (and DERIV_SHIFT) ride row 256 (multiplied by an on-chip ones row)
    W2e = W2 * eta
    b2e = b2 * eta
    cols = np.arange(3 * K * DH).reshape(DH, 3, K)
    uw_cols = cols[:, 0, :].reshape(-1)
    uh_cols = cols[:, 1, :].reshape(-1)
    ud_cols = cols[:, 2, :]
    udx_cols = np.concatenate([ud_cols, ud_cols[:, :1]], 1).reshape(-1)
    w2p = np.empty((H + 1, 776), np.float32)
    w2p[:H, 0:256] = W2e[:, uw_cols]
    w2p[:H, 256:512] = W2e[:, uh_cols]
    w2p[:H, 512:776] = W2e[:, udx_cols]
    w2p[H, 0:256] = b2e[uw_cols]
    w2p[H, 256:512] = b2e[uh_cols]
    w2p[H, 512:776] = b2e[udx_cols] + DERIV_SHIFT
    b1r = np.ascontiguousarray(b1.reshape(2, 128).T)
    a_w = 1.0 - MIN_W * K
    basis = np.zeros((16, 256), np.float32)
    for d in range(DH):
        basis[d, 32 * d:32 * (d + 1)] = 1.0
        basis[8 + d, 32 * d:32 * (d + 1)] = -(np.arange(K) + 1) * MIN_W / a_w
    ident = np.eye(128, dtype=np.float32)

    in_maps = []
    for c in range(NCORES):
        sl = slice(c * bc, (c + 1) * bc)
        in_maps.append(dict(
            theta=theta[sl],
            xT=np.ascontiguousarray(x[sl].T),
            w1=W1, b1=b1r, w2=w2p, basis=basis, ident=ident))
    return in_maps


def kernel(theta, x_conditioner, W1, b1, W2, b2, eta):
    B = np.asarray(theta).shape[0]
    bc = B // NCORES
    nc = build_kernel(bc)
    in_maps = prep_in_maps(theta, x_conditioner, W1, b1, W2, b2, eta)
    res = run_bass_kernel_spmd(nc, in_maps, core_ids=list(range(NCORES)))
    outs = np.concatenate([r["outs"] for r in res.results], 0)
    lads = np.concatenate([r["lad"] for r in res.results], 0)
    return outs, lads
